# revision 8
# baseline (speedup 1.0000x reference)
"""Quantized 3x3 conv (int8-style QAT conv) on 8 TRN2 NeuronCores.

Reference semantics:
    qx = clip(round(x * (127/3)), -127, 127)          # int values in f32
    qw = clip(round(w * (127/0.05)), -127, 127)
    out = conv2d(qx, qw, stride 1, pad 1) * (3*0.05/127^2) + bias[None,:,None,None]

Strategy: pure data parallelism over batch (32 images -> 4 per core), no
collectives. Quantized values are integers <= 127, which are exact in bf16,
so the conv runs as bf16 matmuls with fp32 PSUM accumulation (bit-accurate
int arithmetic). Per core:
  - Activations are quantized on DVE only (tensor_scalar x3: mult+magic-add
    for round-to-nearest-even, then clamps) into zero-padded bf16 tiles,
    split into top/bottom halves per image so the first matmuls start after
    only half an image is resident. ScalarE must stay out of this chain:
    its FIFO is filled with epilogues and would stall the PE.
  - Weights ship as [tap, ci, co] (host does the pure layout permute), are
    DMA'd chunk-by-chunk and quantized via ScalarE round + GpSimd clamps.
  - The 3x3 conv = 9 shifted bf16 matmuls accumulated in PSUM. For each
    group of 4 row-tiles (8 rows x 56 cols, N=448 <= one PSUM bank) and
    each 128-wide cout chunk: out[co, y, x] += qw[tap][ci, co].T @
    qx[ci, y+dy, x+dx], weights loaded once per 4 matmuls.
  - ~5us of dummy matmuls bridge the input-pipeline head so the PE's HAM
    clock gate is already at 8/8 (2.4 GHz) when the real stream begins.
  - Epilogues (rescale + bias from PSUM) ride ScalarE so VectorE keeps its
    budget for quantization (the second-busiest engine at full PE rate); the
    final two groups alternate ScalarE/VectorE to parallelize the tail.
    Output DMAs use the ACT HWDGE ring, input loads the SP ring.
Measured steady state (paired-slope through the axon tunnel): ~45-55us/core
in unloaded windows, ~75us under moderate external load on the shared chip
(PE-stream bound either way; the 504 N=448 matmuls are gapless after an
~8.5us data-latency head, with a ~4.5us drain tail).
"""

import numpy as np

import concourse.mybir as mybir
import concourse.tile as tile
from concourse import bacc
from concourse.bass_utils import run_bass_kernel_spmd

# Problem constants
B, CIN, COUT, H, W, KS = 32, 128, 256, 56, 56, 3
NCORES = 8
BPC = B // NCORES          # images per core
NPIX = H * W               # 3136
HP = H + 2                 # padded spatial
QL = 127.0
SX = QL / 3.0              # activation quant scale
SW = QL / 0.05             # weight quant scale
RESCALE = (3.0 * 0.05) / (QL * QL)
MAGIC = 1.5 * 2.0**23      # fp32 round-to-nearest-even trick

ROWS = 8                   # output rows per matmul tile
RT = H // ROWS             # 7 row tiles per image
NTAP = KS * KS
NCHUNK = COUT // 128       # 2 cout chunks
GROUP = 4                  # psum tiles sharing one weight load

F32 = mybir.dt.float32
BF16 = mybir.dt.bfloat16
FP16 = mybir.dt.float16

_NC = None


def _build(reps: int = 1, no_in: bool = False, no_out: bool = False,
           no_mm: bool = False, no_quant: bool = False, quant_mode: int = 5,
           trim: bool = True):
    """Build the SPMD graph. reps>1 wraps the whole per-call pipeline in a
    hardware For loop — used only by the timing harness (bench.py) to
    measure per-iteration HW time through the high-latency tunnel.
    no_in/no_out/no_mm ablate pipeline stages for bottleneck hunting."""
    nc = bacc.Bacc("TRN2", target_bir_lowering=False, num_devices=NCORES)

    x_t = nc.dram_tensor("x", [BPC, CIN, NPIX], F32, kind="ExternalInput")
    w_t = nc.dram_tensor("weight", [NTAP, CIN, COUT], F32, kind="ExternalInput")
    b_t = nc.dram_tensor("bias", [NCHUNK, 128, 1], F32, kind="ExternalInput")
    o_t = nc.dram_tensor("out", [BPC, NCHUNK, 128, NPIX], F32, kind="ExternalOutput")

    with tile.TileContext(nc) as tc:
        with (
            tc.tile_pool(name="consts", bufs=1) as consts,
            tc.tile_pool(name="xq", bufs=1) as xqp,
            tc.tile_pool(name="xstage", bufs=2) as xsp,
            tc.tile_pool(name="tmp", bufs=2) as tmpp,
            tc.tile_pool(name="outp", bufs=6) as outp,
            tc.tile_pool(name="psum", bufs=8, space="PSUM") as psp,
        ):
            # ---- padded quantized activations: top/bottom half tiles per
            # image. Split tiles give the matmuls finer-grained deps, so the
            # first groups start after only half an image is quantized. Only
            # the pad borders are memset (the interior is fully overwritten).
            # top tile = padded rows 0..33, bottom tile = padded rows 32..57.
            TROWS, BROWS = 34, 26
            xqt, xqb = [], []
            for b in range(BPC):
                tt = xqp.tile([128, TROWS, HP], BF16, tag=f"xqt{b}")
                bt = xqp.tile([128, BROWS, HP], BF16, tag=f"xqb{b}")
                nc.gpsimd.memset(tt[:, 0, :], 0.0)
                nc.gpsimd.memset(tt[:, 1:TROWS, 0], 0.0)
                nc.gpsimd.memset(tt[:, 1:TROWS, HP - 1], 0.0)
                nc.gpsimd.memset(bt[:, BROWS - 1, :], 0.0)
                nc.gpsimd.memset(bt[:, 0 : BROWS - 1, 0], 0.0)
                nc.gpsimd.memset(bt[:, 0 : BROWS - 1, HP - 1], 0.0)
                xqt.append(tt)
                xqb.append(bt)

            # ---- weights: DMA [ci, tap, co] on the ACT HWDGE ring (parallel
            # with x loads on the SP ring), quantize per cout-chunk: ACT does
            # the scale+round, GpSimd does the clamps so DVE stays free for
            # activation quant ----
            wq = consts.tile([128, NTAP, COUT], BF16, tag="wq")
            for c in range(NCHUNK):
                wraw = consts.tile([128, NTAP, 128], F32, tag=f"wraw{c}")
                weng = nc.sync if c == 0 else nc.scalar
                weng.dma_start(
                    out=wraw[:],
                    in_=w_t[:, :, c * 128:(c + 1) * 128].rearrange("t p c -> p t c"),
                )
                wtmp = consts.tile([128, NTAP, 128], F32, tag=f"wtmp{c}")
                nc.scalar.activation(
                    wtmp[:], wraw[:], mybir.ActivationFunctionType.Copy,
                    bias=MAGIC, scale=SW,
                )
                nc.gpsimd.tensor_scalar(
                    wtmp[:], wtmp[:], MAGIC, -QL,
                    mybir.AluOpType.subtract, mybir.AluOpType.max,
                )
                nc.gpsimd.tensor_scalar_min(
                    wq[:, :, c * 128:(c + 1) * 128], wtmp[:], QL)

            bias_sb = []
            for c in range(NCHUNK):
                bs = consts.tile([128, 1], F32, tag=f"bias{c}")
                nc.scalar.dma_start(out=bs[:], in_=b_t[c])
                bias_sb.append(bs)

            # ---- PE warmup: ~5us of dummy matmuls starting at t~0 flips the
            # HAM clock gate to 8/8 before the real matmuls begin (the PE is
            # idle during the input/weight pipelines anyway) ----
            warm = consts.tile([128, 512], BF16, tag="warm")
            nc.gpsimd.memset(warm[:], 1.0)
            wpt = psp.tile([128, 512], F32, tag="pt", name="warm_pt")
            for i in range(14):
                nc.tensor.matmul(wpt[:], warm[:, 0:128], warm[:, 0:512],
                                 start=True, stop=True)

            def body(_iv=None):
                # (x row0, nrows, dst list, dst row offset) for the halves:
                # top interior rows 1..33 <- x rows 0..32; bottom local rows
                # 0..24 <- x rows 31..55 (rows 31..32 quantized twice).
                halves = [(0, TROWS - 1, xqt, 1), (31, H - 31, xqb, 0)]
                for b in range(BPC) if not no_in else []:
                    for hi, (row0, nrows, dst_list, drow) in enumerate(halves):
                        xs = xsp.tile([128, nrows * W], F32, tag=f"xs{hi}",
                                      name=f"xs{b}_{hi}")
                        nc.sync.dma_start(
                            out=xs[:],
                            in_=x_t[b, :, row0 * W : (row0 + nrows) * W])
                        if no_quant:
                            continue
                        # fp16 magic-round chain: fp16(x*SX + 1536) rounds to
                        # the nearest int (ulp=1 in [1024,2048)), clamps ride
                        # the shifted domain, and the final subtract restores
                        # qx into the padded bf16 tile. 2-byte ops halve DVE
                        # cycles and SBUF traffic vs the f32 chain (same
                        # values except ~6e-5 of ties, +-1, from double
                        # rounding -- adds ~2e-4 rel err).
                        t1 = tmpp.tile([128, nrows * W], FP16, tag=f"t1_{hi}",
                                       name=f"t1_{b}_{hi}")
                        nc.vector.tensor_scalar(
                            t1[:], xs[:], SX, 1536.0,
                            mybir.AluOpType.mult, mybir.AluOpType.add,
                        )
                        nc.vector.tensor_scalar(
                            t1[:], t1[:], 1536.0 - QL, 1536.0 + QL,
                            mybir.AluOpType.max, mybir.AluOpType.min,
                        )
                        nc.vector.tensor_scalar_sub(
                            dst_list[b][:, drow : drow + nrows, 1 : W + 1],
                            t1[:].rearrange("p (h w) -> p h w", h=nrows),
                            1536.0,
                        )

                # ---- conv: 9 shifted matmuls accumulated in PSUM ----
                tiles = [] if no_mm else [(b, r) for b in range(BPC) for r in range(RT)]
                sizes = [GROUP] * (len(tiles) // GROUP - 1) + [GROUP - 1, 1] \
                    if tiles else []
                bounds = [0]
                for s in sizes:
                    bounds.append(bounds[-1] + s)
                # Center tap first: it always covers the full 8x56 tile, so
                # start=True zeroes the whole PSUM region before the trimmed
                # edge taps accumulate partial windows into offset views.
                TAP_ORDER = [4, 0, 1, 2, 3, 5, 6, 7, 8]
                for g in range(len(sizes)):
                    grp = tiles[bounds[g] : bounds[g + 1]]
                    for c in range(NCHUNK):
                        pts = [
                            psp.tile([128, ROWS * W], F32, tag="pt",
                                     name=f"pt{g}_{c}_{i}")
                            for i, _ in enumerate(grp)
                        ]
                        for ti, tap in enumerate(TAP_ORDER):
                            ky, kx = divmod(tap, KS)
                            lhsT = wq[:, tap, c * 128 : (c + 1) * 128]
                            for t, (b, r) in enumerate(grp):
                                prow = r * ROWS + ky
                                # trim rows/cols of the window that read only
                                # zero padding (top/bottom image rows, left/
                                # right pad columns) and accumulate into the
                                # matching offset view of the PSUM tile
                                # row-trim only: windows whose first/last row
                                # is all zero padding stream 7 rows instead of
                                # 8, writing a flat offset PSUM slice (keeps
                                # 2D out APs; col-trim would force segmented
                                # 3D PSUM writes that cost more than the 0.2%
                                # of columns they save)
                                orow, nr = 0, ROWS
                                if trim:
                                    if r == 0 and ky == 0:
                                        orow, nr = 1, ROWS - 1
                                    if r == RT - 1 and ky == 2:
                                        nr = ROWS - 1
                                wr = prow + orow
                                if r < 4:
                                    rhs = xqt[b][:, wr : wr + nr, kx : kx + W]
                                else:
                                    rhs = xqb[b][:, wr - 32 : wr - 32 + nr,
                                                 kx : kx + W]
                                out = pts[t][:, orow * W : (orow + nr) * W]
                                nc.tensor.matmul(
                                    out, lhsT, rhs,
                                    start=(ti == 0), stop=(ti == NTAP - 1),
                                )
                        # epilogues ride ScalarE so VectorE keeps its budget
                        # for quantization (DVE is the second-busiest engine
                        # when the PE streams at full rate); the final two
                        # groups alternate ACT/DVE to parallelize the tail
                        tail_grp = g >= len(sizes) - 2
                        for t, (b, r) in enumerate(grp):
                            ot = outp.tile([128, ROWS * W], F32, tag="ot",
                                           name=f"ot{g}_{c}_{t}")
                            if not tail_grp or t % 2 == 0:
                                nc.scalar.activation(
                                    ot[:], pts[t][:],
                                    mybir.ActivationFunctionType.Identity,
                                    bias=bias_sb[c][:], scale=RESCALE,
                                )
                            else:
                                nc.vector.tensor_scalar(
                                    ot[:], pts[t][:], RESCALE, bias_sb[c][:],
                                    mybir.AluOpType.mult, mybir.AluOpType.add,
                                )
                            if not no_out:
                                nc.scalar.dma_start(
                                    out=o_t[b, c, :, r * ROWS * W : (r + 1) * ROWS * W],
                                    in_=ot[:],
                                )

            if reps == 1:
                body()
            else:
                with tc.For_i(0, reps, 1):
                    body()
    nc.compile()
    return nc


def _get_nc():
    global _NC
    if _NC is None:
        _NC = _build()
    return _NC


def prep_in_maps(x: np.ndarray, weight: np.ndarray, bias: np.ndarray):
    """Host-side layout permutes only: full inputs -> per-core input dicts."""
    x = np.ascontiguousarray(np.asarray(x), dtype=np.float32).reshape(B, CIN, NPIX)
    # pure layout permute: [co, ci, ky, kx] -> [ky*kx, ci, co]
    w_l = np.ascontiguousarray(
        np.asarray(weight, dtype=np.float32).transpose(2, 3, 1, 0)
    ).reshape(NTAP, CIN, COUT)
    b_l = np.ascontiguousarray(
        np.asarray(bias, dtype=np.float32)).reshape(NCHUNK, 128, 1)
    return [
        {
            "x": np.ascontiguousarray(x[i * BPC : (i + 1) * BPC]),
            "weight": w_l,
            "bias": b_l,
        }
        for i in range(NCORES)
    ]


def kernel(x: np.ndarray, weight: np.ndarray, bias: np.ndarray) -> np.ndarray:
    """Full inputs in, full output out. Shards batch across 8 cores."""
    nc = _get_nc()
    in_maps = prep_in_maps(x, weight, bias)
    res = run_bass_kernel_spmd(nc, in_maps, core_ids=list(range(NCORES)))
    out = np.concatenate(
        [r["out"].reshape(BPC, COUT, H, W) for r in res.results], axis=0
    )
    return out



# revision 15
# speedup vs baseline: 1.9514x; 1.9514x over previous
"""Quantized 3x3 conv (int8-style QAT conv) on 8 TRN2 NeuronCores.

Reference semantics:
    qx = clip(round(x * (127/3)), -127, 127)          # int values in f32
    qw = clip(round(w * (127/0.05)), -127, 127)
    out = conv2d(qx, qw, stride 1, pad 1) * (3*0.05/127^2) + bias[None,:,None,None]

Strategy: pure data parallelism over batch (32 images -> 4 per core), no
collectives. Quantized values are integers <= 127, which are exact in bf16,
so the conv runs as bf16 matmuls with fp32 PSUM accumulation (bit-accurate
int arithmetic). Per core:
  - Activations are quantized on DVE only (tensor_scalar x3: mult+magic-add
    for round-to-nearest-even, then clamps) into zero-padded bf16 tiles,
    split into top/bottom halves per image so the first matmuls start after
    only half an image is resident. ScalarE must stay out of this chain:
    its FIFO is filled with epilogues and would stall the PE.
  - Weights ship as [tap, ci, co] (host does the pure layout permute), are
    DMA'd chunk-by-chunk and quantized via ScalarE round + GpSimd clamps.
  - The 3x3 conv = 9 shifted bf16 matmuls accumulated in PSUM. For each
    group of 4 row-tiles (8 rows x 56 cols, N=448 <= one PSUM bank) and
    each 128-wide cout chunk: out[co, y, x] += qw[tap][ci, co].T @
    qx[ci, y+dy, x+dx], weights loaded once per 4 matmuls.
  - ~5us of dummy matmuls bridge the input-pipeline head so the PE's HAM
    clock gate is already at 8/8 (2.4 GHz) when the real stream begins.
  - Epilogues (rescale + bias from PSUM) ride ScalarE so VectorE keeps its
    budget for quantization (the second-busiest engine at full PE rate); the
    final two groups alternate ScalarE/VectorE to parallelize the tail.
    Output DMAs use the ACT HWDGE ring, input loads the SP ring.
Measured steady state (paired-slope through the axon tunnel): ~45-55us/core
in unloaded windows, ~75us under moderate external load on the shared chip
(PE-stream bound either way; the 504 N=448 matmuls are gapless after an
~8.5us data-latency head, with a ~4.5us drain tail).
"""

import numpy as np

import concourse.mybir as mybir
import concourse.tile as tile
from concourse import bacc
from concourse.bass_utils import run_bass_kernel_spmd

# Problem constants
B, CIN, COUT, H, W, KS = 32, 128, 256, 56, 56, 3
NCORES = 8
BPC = B // NCORES          # images per core
NPIX = H * W               # 3136
HP = H + 2                 # padded spatial
QL = 127.0
SX = QL / 3.0              # activation quant scale
SW = QL / 0.05             # weight quant scale
RESCALE = (3.0 * 0.05) / (QL * QL)
MAGIC = 1.5 * 2.0**23      # fp32 round-to-nearest-even trick

ROWS = 8                   # output rows per matmul tile
RT = H // ROWS             # 7 row tiles per image
NTAP = KS * KS
NCHUNK = COUT // 128       # 2 cout chunks
GROUP = 4                  # psum tiles sharing one weight load

F32 = mybir.dt.float32
BF16 = mybir.dt.bfloat16
FP16 = mybir.dt.float16

_NC = None


def _build(reps: int = 1, no_in: bool = False, no_out: bool = False,
           no_mm: bool = False, no_quant: bool = False, quant_mode: int = 5,
           trim: bool = True, fp16q: bool = True):
    """Build the SPMD graph. reps>1 wraps the whole per-call pipeline in a
    hardware For loop — used only by the timing harness (bench.py) to
    measure per-iteration HW time through the high-latency tunnel.
    no_in/no_out/no_mm ablate pipeline stages for bottleneck hunting."""
    nc = bacc.Bacc("TRN2", target_bir_lowering=False, num_devices=NCORES)

    x_t = nc.dram_tensor("x", [BPC, CIN, NPIX], FP16, kind="ExternalInput")
    w_t = nc.dram_tensor("weight", [NTAP, CIN, COUT], F32, kind="ExternalInput")
    b_t = nc.dram_tensor("bias", [NCHUNK, 128, 1], F32, kind="ExternalInput")
    o_t = nc.dram_tensor("out", [BPC, NCHUNK, 128, NPIX], F32, kind="ExternalOutput")
    SHIFT = 1536.0             # fp16 magic-round offset (ulp=1 in [1024,2048))

    with tile.TileContext(nc) as tc:
        with (
            tc.tile_pool(name="consts", bufs=1) as consts,
            tc.tile_pool(name="xq", bufs=1) as xqp,
            tc.tile_pool(name="xstage", bufs=2) as xsp,
            tc.tile_pool(name="tmp", bufs=2) as tmpp,
            tc.tile_pool(name="outp", bufs=6) as outp,
            tc.tile_pool(name="psum", bufs=8, space="PSUM") as psp,
        ):
            # ---- padded quantized activations: top/bottom half tiles per
            # image. Split tiles give the matmuls finer-grained deps, so the
            # first groups start after only half an image is quantized. Only
            # the pad borders are memset (the interior is fully overwritten).
            # top tile = padded rows 0..33, bottom tile = padded rows 32..57.
            # Activations live in the shifted domain s = qx + SHIFT (exact
            # ints in fp16), so quantization is two DVE ops and the matmul
            # accumulates out + SHIFT*sum(qw); the constant is removed via
            # effective biases computed in the head. Pad cells hold SHIFT
            # (shifted-domain zero).
            TROWS, BROWS = 34, 26
            xqt, xqb = [], []
            for b in range(BPC):
                tt = xqp.tile([128, TROWS, HP], FP16, tag=f"xqt{b}")
                bt = xqp.tile([128, BROWS, HP], FP16, tag=f"xqb{b}")
                nc.gpsimd.memset(tt[:, 0, :], SHIFT)
                nc.gpsimd.memset(tt[:, 1:TROWS, 0], SHIFT)
                nc.gpsimd.memset(tt[:, 1:TROWS, HP - 1], SHIFT)
                nc.gpsimd.memset(bt[:, BROWS - 1, :], SHIFT)
                nc.gpsimd.memset(bt[:, 0 : BROWS - 1, 0], SHIFT)
                nc.gpsimd.memset(bt[:, 0 : BROWS - 1, HP - 1], SHIFT)
                xqt.append(tt)
                xqb.append(bt)

            # ---- weights: DMA [ci, tap, co] on the ACT HWDGE ring (parallel
            # with x loads on the SP ring), quantize per cout-chunk: ACT does
            # the scale+round, GpSimd does the clamps so DVE stays free for
            # activation quant ----
            wq = consts.tile([128, NTAP, COUT], FP16, tag="wq")
            for c in range(NCHUNK):
                wraw = consts.tile([128, NTAP, 128], F32, tag=f"wraw{c}")
                weng = nc.sync if c == 0 else nc.scalar
                weng.dma_start(
                    out=wraw[:],
                    in_=w_t[:, :, c * 128:(c + 1) * 128].rearrange("t p c -> p t c"),
                )
                wtmp = consts.tile([128, NTAP, 128], F32, tag=f"wtmp{c}")
                nc.scalar.activation(
                    wtmp[:], wraw[:], mybir.ActivationFunctionType.Copy,
                    bias=MAGIC, scale=SW,
                )
                nc.gpsimd.tensor_scalar(
                    wtmp[:], wtmp[:], MAGIC, -QL,
                    mybir.AluOpType.subtract, mybir.AluOpType.max,
                )
                nc.gpsimd.tensor_scalar_min(
                    wq[:, :, c * 128:(c + 1) * 128], wtmp[:], QL)

            bias_sb = []
            for c in range(NCHUNK):
                bs = consts.tile([128, 1], F32, tag=f"bias{c}")
                nc.scalar.dma_start(out=bs[:], in_=b_t[c])
                bias_sb.append(bs)

            # ---- shifted-domain compensation: S_ky[co] = sum_ci,kx qw for
            # each tap row, via 3-tap matmul accumulation against ones.
            # bias_full = bias - RESCALE*SHIFT*(S0+S1+S2); output row 0 lacks
            # the (trimmed) ky=0 taps -> bias_r0 = bias - K*(S1+S2); row 55
            # lacks ky=2 -> bias_r6 = bias - K*(S0+S1). ----
            ones_sb = consts.tile([128, 1], FP16, tag="ones")
            nc.gpsimd.memset(ones_sb[:], 1.0)
            KSH = RESCALE * SHIFT
            bias_full, bias_r0, bias_r6 = [], [], []
            for c in range(NCHUNK):
                s_sb = []
                for ky in range(KS):
                    sp = psp.tile([128, 1], F32, tag="pt", name=f"s_ps{c}_{ky}")
                    for kx in range(KS):
                        nc.tensor.matmul(
                            sp[:], wq[:, ky * KS + kx, c * 128:(c + 1) * 128],
                            ones_sb[:], start=(kx == 0), stop=(kx == KS - 1),
                        )
                    sb = consts.tile([128, 1], F32, tag=f"s_sb{c}_{ky}")
                    nc.vector.tensor_scalar_add(sb[:], sp[:], 0.0)
                    s_sb.append(sb)
                s01 = consts.tile([128, 1], F32, tag=f"s01_{c}")
                s12 = consts.tile([128, 1], F32, tag=f"s12_{c}")
                nc.gpsimd.tensor_tensor(s01[:], s_sb[0][:], s_sb[1][:],
                                        mybir.AluOpType.add)
                nc.gpsimd.tensor_tensor(s12[:], s_sb[1][:], s_sb[2][:],
                                        mybir.AluOpType.add)
                for tag, parts, dst in (
                    (f"bf_{c}", (s01, s_sb[2]), bias_full),
                    (f"b0_{c}", (s12, None), bias_r0),
                    (f"b6_{c}", (s01, None), bias_r6),
                ):
                    tot = consts.tile([128, 1], F32, tag=f"t{tag}")
                    if parts[1] is not None:
                        nc.gpsimd.tensor_tensor(tot[:], parts[0][:],
                                                parts[1][:],
                                                mybir.AluOpType.add)
                    else:
                        nc.gpsimd.tensor_scalar_add(tot[:], parts[0][:], 0.0)
                    be = consts.tile([128, 1], F32, tag=f"e{tag}")
                    nc.gpsimd.tensor_scalar(
                        be[:], tot[:], -KSH, None, mybir.AluOpType.mult)
                    nc.gpsimd.tensor_tensor(be[:], be[:], bias_sb[c][:],
                                            mybir.AluOpType.add)
                    dst.append(be)

            # ---- PE warmup: ~5us of dummy matmuls starting at t~0 flips the
            # HAM clock gate to 8/8 before the real matmuls begin (the PE is
            # idle during the input/weight pipelines anyway) ----
            warm = consts.tile([128, 512], BF16, tag="warm")
            nc.gpsimd.memset(warm[:], 1.0)
            wpt = psp.tile([128, 512], F32, tag="pt", name="warm_pt")
            for i in range(14):
                nc.tensor.matmul(wpt[:], warm[:, 0:128], warm[:, 0:512],
                                 start=True, stop=True)

            def body(_iv=None):
                # (x row0, nrows, dst list, dst row offset) for the halves:
                # top interior rows 1..33 <- x rows 0..32; bottom local rows
                # 0..24 <- x rows 31..55 (rows 31..32 quantized twice).
                halves = [(0, TROWS - 1, xqt, 1), (31, H - 31, xqb, 0)]
                for b in range(BPC) if not no_in else []:
                    for hi, (row0, nrows, dst_list, drow) in enumerate(halves):
                        xs = xsp.tile([128, nrows * W], FP16, tag=f"xs{hi}",
                                      name=f"xs{b}_{hi}")
                        nc.sync.dma_start(
                            out=xs[:],
                            in_=x_t[b, :, row0 * W : (row0 + nrows) * W])
                        if no_quant:
                            continue
                        # two-op shifted-domain quant: fp16(x*SX + SHIFT)
                        # rounds to the int grid (ulp=1 in [1024,2048)), then
                        # a fused max/min clamps to [SHIFT-127, SHIFT+127]
                        # straight into the padded tile. All operands 2-byte.
                        t1 = tmpp.tile([128, nrows * W], FP16, tag=f"t1_{hi}",
                                       name=f"t1_{b}_{hi}")
                        nc.vector.tensor_scalar(
                            t1[:], xs[:], SX, SHIFT,
                            mybir.AluOpType.mult, mybir.AluOpType.add,
                        )
                        nc.vector.tensor_scalar(
                            dst_list[b][:, drow : drow + nrows, 1 : W + 1],
                            t1[:].rearrange("p (h w) -> p h w", h=nrows),
                            SHIFT - QL, SHIFT + QL,
                            mybir.AluOpType.max, mybir.AluOpType.min,
                        )

                # ---- conv: 9 shifted matmuls accumulated in PSUM ----
                tiles = [] if no_mm else [(b, r) for b in range(BPC) for r in range(RT)]
                sizes = [GROUP] * (len(tiles) // GROUP - 1) + [GROUP - 1, 1] \
                    if tiles else []
                bounds = [0]
                for s in sizes:
                    bounds.append(bounds[-1] + s)
                # Center tap first: it always covers the full 8x56 tile, so
                # start=True zeroes the whole PSUM region before the trimmed
                # edge taps accumulate partial windows into offset views.
                TAP_ORDER = [4, 0, 1, 2, 3, 5, 6, 7, 8]
                for g in range(len(sizes)):
                    grp = tiles[bounds[g] : bounds[g + 1]]
                    for c in range(NCHUNK):
                        pts = [
                            psp.tile([128, ROWS * W], F32, tag="pt",
                                     name=f"pt{g}_{c}_{i}")
                            for i, _ in enumerate(grp)
                        ]
                        for ti, tap in enumerate(TAP_ORDER):
                            ky, kx = divmod(tap, KS)
                            lhsT = wq[:, tap, c * 128 : (c + 1) * 128]
                            for t, (b, r) in enumerate(grp):
                                prow = r * ROWS + ky
                                # trim rows/cols of the window that read only
                                # zero padding (top/bottom image rows, left/
                                # right pad columns) and accumulate into the
                                # matching offset view of the PSUM tile
                                # row-trim only: windows whose first/last row
                                # is all zero padding stream 7 rows instead of
                                # 8, writing a flat offset PSUM slice (keeps
                                # 2D out APs; col-trim would force segmented
                                # 3D PSUM writes that cost more than the 0.2%
                                # of columns they save)
                                orow, nr = 0, ROWS
                                if trim:
                                    if r == 0 and ky == 0:
                                        orow, nr = 1, ROWS - 1
                                    if r == RT - 1 and ky == 2:
                                        nr = ROWS - 1
                                wr = prow + orow
                                if r < 4:
                                    rhs = xqt[b][:, wr : wr + nr, kx : kx + W]
                                else:
                                    rhs = xqb[b][:, wr - 32 : wr - 32 + nr,
                                                 kx : kx + W]
                                out = pts[t][:, orow * W : (orow + nr) * W]
                                nc.tensor.matmul(
                                    out, lhsT, rhs,
                                    start=(ti == 0), stop=(ti == NTAP - 1),
                                )
                        # epilogues ride ScalarE so VectorE keeps its budget
                        # for quantization (DVE is the second-busiest engine
                        # when the PE streams at full rate); the final two
                        # groups alternate ACT/DVE to parallelize the tail
                        tail_grp = g >= len(sizes) - 2
                        for t, (b, r) in enumerate(grp):
                            ot = outp.tile([128, ROWS * W], F32, tag="ot",
                                           name=f"ot{g}_{c}_{t}")
                            # edge row-tiles: the trimmed tap rows never saw
                            # the SHIFT-pad contribution, so their first/last
                            # output row takes a different effective bias
                            if trim and r == 0:
                                segs = [(0, W, bias_r0[c]),
                                        (W, ROWS * W, bias_full[c])]
                            elif trim and r == RT - 1:
                                segs = [(0, (ROWS - 1) * W, bias_full[c]),
                                        ((ROWS - 1) * W, ROWS * W, bias_r6[c])]
                            else:
                                segs = [(0, ROWS * W, bias_full[c])]
                            for s0, s1, bseg in segs:
                                if not tail_grp or t % 2 == 0:
                                    nc.scalar.activation(
                                        ot[:, s0:s1], pts[t][:, s0:s1],
                                        mybir.ActivationFunctionType.Identity,
                                        bias=bseg[:], scale=RESCALE,
                                    )
                                else:
                                    nc.vector.tensor_scalar(
                                        ot[:, s0:s1], pts[t][:, s0:s1],
                                        RESCALE, bseg[:],
                                        mybir.AluOpType.mult,
                                        mybir.AluOpType.add,
                                    )
                            if not no_out:
                                nc.scalar.dma_start(
                                    out=o_t[b, c, :, r * ROWS * W : (r + 1) * ROWS * W],
                                    in_=ot[:],
                                )

            if reps == 1:
                body()
            else:
                with tc.For_i(0, reps, 1):
                    body()
    nc.compile()
    return nc


def _get_nc():
    global _NC
    if _NC is None:
        _NC = _build()
    return _NC


def prep_in_maps(x: np.ndarray, weight: np.ndarray, bias: np.ndarray):
    """Host-side layout permutes + fp16 downcast of x (the quantizer only
    needs ~8 significant bits; fp16's 11 keep the off-by-one rate ~2%)."""
    x = np.ascontiguousarray(np.asarray(x), dtype=np.float32).reshape(
        B, CIN, NPIX).astype(np.float16)
    # pure layout permute: [co, ci, ky, kx] -> [ky*kx, ci, co]
    w_l = np.ascontiguousarray(
        np.asarray(weight, dtype=np.float32).transpose(2, 3, 1, 0)
    ).reshape(NTAP, CIN, COUT)
    b_l = np.ascontiguousarray(
        np.asarray(bias, dtype=np.float32)).reshape(NCHUNK, 128, 1)
    return [
        {
            "x": np.ascontiguousarray(x[i * BPC : (i + 1) * BPC]),
            "weight": w_l,
            "bias": b_l,
        }
        for i in range(NCORES)
    ]


def kernel(x: np.ndarray, weight: np.ndarray, bias: np.ndarray) -> np.ndarray:
    """Full inputs in, full output out. Shards batch across 8 cores."""
    nc = _get_nc()
    in_maps = prep_in_maps(x, weight, bias)
    res = run_bass_kernel_spmd(nc, in_maps, core_ids=list(range(NCORES)))
    out = np.concatenate(
        [r["out"].reshape(BPC, COUT, H, W) for r in res.results], axis=0
    )
    return out



# revision 18
# speedup vs baseline: 2.4916x; 1.2768x over previous
"""Quantized 3x3 conv (int8-style QAT conv) on 8 TRN2 NeuronCores.

Reference semantics:
    qx = clip(round(x * (127/3)), -127, 127)          # int values in f32
    qw = clip(round(w * (127/0.05)), -127, 127)
    out = conv2d(qx, qw, stride 1, pad 1) * (3*0.05/127^2) + bias[None,:,None,None]

Strategy: pure data parallelism over batch (32 images -> 4 per core), no
collectives. Quantized values are integers <= 127, exact in fp16, so the
conv runs as fp16 matmuls with fp32 PSUM accumulation (bit-accurate int
arithmetic). Per core:
  - x ships as fp16 (halves input DMA; costs ~1e-3 rel err from off-by-one
    rounding of the quantizer at bin edges).
  - Activations are kept in the SHIFTED domain s = qx + 1536: two fused DVE
    tensor_scalar ops (fp16 magic-round mult+add, then max/min clamp) write
    straight into 1536-padded fp16 tiles -- all operands 2-byte, no
    restore-subtract pass. The matmul then accumulates out + 1536*sum(qw);
    that constant is folded into effective per-channel biases computed in
    the head from S_ky[co] = sum_{ci,kx} qw (3 tap-row matmuls against a
    ones vector, per cout chunk). ScalarE stays out of the quant chain: its
    FIFO is filled with epilogues and would stall the PE.
  - Weights ship as [tap, ci, co] (host does the pure layout permute), are
    DMA'd chunk-by-chunk and quantized via ScalarE round + GpSimd clamps.
  - The 3x3 conv = 9 shifted fp16 matmuls accumulated in PSUM. For each
    group of 4 row-tiles (8 rows x 56 cols, N=448 <= one PSUM bank) and
    each 128-wide cout chunk: out[co, y, x] += qw[tap][ci, co].T @
    s[ci, y+dy, x+dx], weights loaded once per 4 matmuls. Edge windows
    whose first/last row reads only padding are row-trimmed (N=392,
    offset PSUM view) -- saves 1.2% of the PE stream; the center tap runs
    first with start=True to zero the full PSUM tile. Output rows that
    lost trimmed taps take adjusted biases (bias_r0/bias_r6).
  - ~5us of dummy matmuls bridge the input-pipeline head so the PE's HAM
    clock gate is already at 8/8 (2.4 GHz) when the real stream begins.
  - Epilogues (rescale + eff-bias from PSUM) ride ScalarE; the final two
    groups alternate ScalarE/VectorE to parallelize the tail. Output DMAs
    use the ACT HWDGE ring, input loads the SP ring.
PE floor: 223,104 streamed columns x ~0.206 ns = ~46.0us/core steady state;
measured gapless in unloaded windows (external load on the shared chip inflates
wall numbers; fp8/DoubleRow and Winograd were measured/analyzed and lose --
fp8 needs 3x the K-tiles at only 2x pump, transforms blow the DVE budget).
"""

import numpy as np

import concourse.mybir as mybir
import concourse.tile as tile
from concourse import bacc
from concourse.bass_utils import run_bass_kernel_spmd

# Problem constants
B, CIN, COUT, H, W, KS = 32, 128, 256, 56, 56, 3
NCORES = 8
BPC = B // NCORES          # images per core
NPIX = H * W               # 3136
HP = H + 2                 # padded spatial
QL = 127.0
SX = QL / 3.0              # activation quant scale
SW = QL / 0.05             # weight quant scale
RESCALE = (3.0 * 0.05) / (QL * QL)
MAGIC = 1.5 * 2.0**23      # fp32 round-to-nearest-even trick

ROWS = 8                   # output rows per matmul tile
RT = H // ROWS             # 7 row tiles per image
NTAP = KS * KS
NCHUNK = COUT // 128       # 2 cout chunks
GROUP = 4                  # psum tiles sharing one weight load

F32 = mybir.dt.float32
BF16 = mybir.dt.bfloat16
FP16 = mybir.dt.float16

_NC = None


def _build(reps: int = 1, no_in: bool = False, no_out: bool = False,
           no_mm: bool = False, no_quant: bool = False, trim: bool = True):
    """Build the SPMD graph. reps>1 wraps the whole per-call pipeline in a
    hardware For loop — used only by the timing harness (bench.py) to
    measure per-iteration HW time through the high-latency tunnel.
    no_in/no_out/no_mm ablate pipeline stages for bottleneck hunting."""
    nc = bacc.Bacc("TRN2", target_bir_lowering=False, num_devices=NCORES)

    x_t = nc.dram_tensor("x", [BPC, CIN, NPIX], FP16, kind="ExternalInput")
    w_t = nc.dram_tensor("weight", [NTAP, CIN, COUT], F32, kind="ExternalInput")
    b_t = nc.dram_tensor("bias", [NCHUNK, 128, 1], F32, kind="ExternalInput")
    o_t = nc.dram_tensor("out", [BPC, NCHUNK, 128, NPIX], F32, kind="ExternalOutput")
    SHIFT = 1536.0             # fp16 magic-round offset (ulp=1 in [1024,2048))

    with tile.TileContext(nc) as tc:
        with (
            tc.tile_pool(name="consts", bufs=1) as consts,
            tc.tile_pool(name="xq", bufs=1) as xqp,
            tc.tile_pool(name="xstage", bufs=2) as xsp,
            tc.tile_pool(name="tmp", bufs=2) as tmpp,
            tc.tile_pool(name="outp", bufs=6) as outp,
            tc.tile_pool(name="psum", bufs=8, space="PSUM") as psp,
        ):
            # ---- padded quantized activations: top/bottom half tiles per
            # image. Split tiles give the matmuls finer-grained deps, so the
            # first groups start after only half an image is quantized. Only
            # the pad borders are memset (the interior is fully overwritten).
            # top tile = padded rows 0..33, bottom tile = padded rows 32..57.
            # Activations live in the shifted domain s = qx + SHIFT (exact
            # ints in fp16), so quantization is two DVE ops and the matmul
            # accumulates out + SHIFT*sum(qw); the constant is removed via
            # effective biases computed in the head. Pad cells hold SHIFT
            # (shifted-domain zero).
            TROWS, BROWS = 34, 26
            xqt, xqb = [], []
            for b in range(BPC):
                tt = xqp.tile([128, TROWS, HP], FP16, tag=f"xqt{b}")
                bt = xqp.tile([128, BROWS, HP], FP16, tag=f"xqb{b}")
                nc.gpsimd.memset(tt[:, 0, :], SHIFT)
                nc.gpsimd.memset(tt[:, 1:TROWS, 0], SHIFT)
                nc.gpsimd.memset(tt[:, 1:TROWS, HP - 1], SHIFT)
                nc.gpsimd.memset(bt[:, BROWS - 1, :], SHIFT)
                nc.gpsimd.memset(bt[:, 0 : BROWS - 1, 0], SHIFT)
                nc.gpsimd.memset(bt[:, 0 : BROWS - 1, HP - 1], SHIFT)
                xqt.append(tt)
                xqb.append(bt)

            # ---- weights: DMA [ci, tap, co] on the ACT HWDGE ring (parallel
            # with x loads on the SP ring), quantize per cout-chunk: ACT does
            # the scale+round, GpSimd does the clamps so DVE stays free for
            # activation quant ----
            wq = consts.tile([128, NTAP, COUT], FP16, tag="wq")
            for c in range(NCHUNK):
                wraw = consts.tile([128, NTAP, 128], F32, tag=f"wraw{c}")
                weng = nc.sync if c == 0 else nc.scalar
                weng.dma_start(
                    out=wraw[:],
                    in_=w_t[:, :, c * 128:(c + 1) * 128].rearrange("t p c -> p t c"),
                )
                wtmp = consts.tile([128, NTAP, 128], F32, tag=f"wtmp{c}")
                nc.scalar.activation(
                    wtmp[:], wraw[:], mybir.ActivationFunctionType.Copy,
                    bias=MAGIC, scale=SW,
                )
                nc.gpsimd.tensor_scalar(
                    wtmp[:], wtmp[:], MAGIC, -QL,
                    mybir.AluOpType.subtract, mybir.AluOpType.max,
                )
                nc.gpsimd.tensor_scalar_min(
                    wq[:, :, c * 128:(c + 1) * 128], wtmp[:], QL)

            bias_sb = []
            for c in range(NCHUNK):
                bs = consts.tile([128, 1], F32, tag=f"bias{c}")
                nc.scalar.dma_start(out=bs[:], in_=b_t[c])
                bias_sb.append(bs)

            # ---- shifted-domain compensation: S_ky[co] = sum_ci,kx qw for
            # each tap row, via 3-tap matmul accumulation against ones.
            # bias_full = bias - RESCALE*SHIFT*(S0+S1+S2); output row 0 lacks
            # the (trimmed) ky=0 taps -> bias_r0 = bias - K*(S1+S2); row 55
            # lacks ky=2 -> bias_r6 = bias - K*(S0+S1). ----
            ones_sb = consts.tile([128, 1], FP16, tag="ones")
            nc.gpsimd.memset(ones_sb[:], 1.0)
            KSH = RESCALE * SHIFT
            bias_full, bias_r0, bias_r6 = [], [], []
            for c in range(NCHUNK):
                s_sb = []
                for ky in range(KS):
                    sp = psp.tile([128, 1], F32, tag="pt", name=f"s_ps{c}_{ky}")
                    for kx in range(KS):
                        nc.tensor.matmul(
                            sp[:], wq[:, ky * KS + kx, c * 128:(c + 1) * 128],
                            ones_sb[:], start=(kx == 0), stop=(kx == KS - 1),
                        )
                    sb = consts.tile([128, 1], F32, tag=f"s_sb{c}_{ky}")
                    nc.vector.tensor_scalar_add(sb[:], sp[:], 0.0)
                    s_sb.append(sb)
                s01 = consts.tile([128, 1], F32, tag=f"s01_{c}")
                s12 = consts.tile([128, 1], F32, tag=f"s12_{c}")
                nc.gpsimd.tensor_tensor(s01[:], s_sb[0][:], s_sb[1][:],
                                        mybir.AluOpType.add)
                nc.gpsimd.tensor_tensor(s12[:], s_sb[1][:], s_sb[2][:],
                                        mybir.AluOpType.add)
                for tag, parts, dst in (
                    (f"bf_{c}", (s01, s_sb[2]), bias_full),
                    (f"b0_{c}", (s12, None), bias_r0),
                    (f"b6_{c}", (s01, None), bias_r6),
                ):
                    tot = consts.tile([128, 1], F32, tag=f"t{tag}")
                    if parts[1] is not None:
                        nc.gpsimd.tensor_tensor(tot[:], parts[0][:],
                                                parts[1][:],
                                                mybir.AluOpType.add)
                    else:
                        nc.gpsimd.tensor_scalar_add(tot[:], parts[0][:], 0.0)
                    be = consts.tile([128, 1], F32, tag=f"e{tag}")
                    nc.gpsimd.tensor_scalar(
                        be[:], tot[:], -KSH, None, mybir.AluOpType.mult)
                    nc.gpsimd.tensor_tensor(be[:], be[:], bias_sb[c][:],
                                            mybir.AluOpType.add)
                    dst.append(be)

            # ---- PE warmup: ~5us of dummy matmuls starting at t~0 flips the
            # HAM clock gate to 8/8 before the real matmuls begin (the PE is
            # idle during the input/weight pipelines anyway) ----
            warm = consts.tile([128, 512], BF16, tag="warm")
            nc.gpsimd.memset(warm[:], 1.0)
            wpt = psp.tile([128, 512], F32, tag="pt", name="warm_pt")
            for i in range(14):
                nc.tensor.matmul(wpt[:], warm[:, 0:128], warm[:, 0:512],
                                 start=True, stop=True)

            def body(_iv=None):
                # (x row0, nrows, dst list, dst row offset) for the halves:
                # top interior rows 1..33 <- x rows 0..32; bottom local rows
                # 0..24 <- x rows 31..55 (rows 31..32 quantized twice).
                halves = [(0, TROWS - 1, xqt, 1), (31, H - 31, xqb, 0)]
                for b in range(BPC) if not no_in else []:
                    for hi, (row0, nrows, dst_list, drow) in enumerate(halves):
                        xs = xsp.tile([128, nrows * W], FP16, tag=f"xs{hi}",
                                      name=f"xs{b}_{hi}")
                        nc.sync.dma_start(
                            out=xs[:],
                            in_=x_t[b, :, row0 * W : (row0 + nrows) * W])
                        if no_quant:
                            continue
                        # two-op shifted-domain quant: fp16(x*SX + SHIFT)
                        # rounds to the int grid (ulp=1 in [1024,2048)), then
                        # a fused max/min clamps to [SHIFT-127, SHIFT+127]
                        # straight into the padded tile. All operands 2-byte.
                        t1 = tmpp.tile([128, nrows * W], FP16, tag=f"t1_{hi}",
                                       name=f"t1_{b}_{hi}")
                        nc.vector.tensor_scalar(
                            t1[:], xs[:], SX, SHIFT,
                            mybir.AluOpType.mult, mybir.AluOpType.add,
                        )
                        nc.vector.tensor_scalar(
                            dst_list[b][:, drow : drow + nrows, 1 : W + 1],
                            t1[:].rearrange("p (h w) -> p h w", h=nrows),
                            SHIFT - QL, SHIFT + QL,
                            mybir.AluOpType.max, mybir.AluOpType.min,
                        )

                # ---- conv: 9 shifted matmuls accumulated in PSUM ----
                tiles = [] if no_mm else [(b, r) for b in range(BPC) for r in range(RT)]
                sizes = [GROUP] * (len(tiles) // GROUP - 1) + [GROUP - 1, 1] \
                    if tiles else []
                bounds = [0]
                for s in sizes:
                    bounds.append(bounds[-1] + s)
                # Center tap first: it always covers the full 8x56 tile, so
                # start=True zeroes the whole PSUM region before the trimmed
                # edge taps accumulate partial windows into offset views.
                TAP_ORDER = [4, 0, 1, 2, 3, 5, 6, 7, 8]
                for g in range(len(sizes)):
                    grp = tiles[bounds[g] : bounds[g + 1]]
                    for c in range(NCHUNK):
                        pts = [
                            psp.tile([128, ROWS * W], F32, tag="pt",
                                     name=f"pt{g}_{c}_{i}")
                            for i, _ in enumerate(grp)
                        ]
                        for ti, tap in enumerate(TAP_ORDER):
                            ky, kx = divmod(tap, KS)
                            lhsT = wq[:, tap, c * 128 : (c + 1) * 128]
                            for t, (b, r) in enumerate(grp):
                                prow = r * ROWS + ky
                                # trim rows/cols of the window that read only
                                # zero padding (top/bottom image rows, left/
                                # right pad columns) and accumulate into the
                                # matching offset view of the PSUM tile
                                # row-trim only: windows whose first/last row
                                # is all zero padding stream 7 rows instead of
                                # 8, writing a flat offset PSUM slice (keeps
                                # 2D out APs; col-trim would force segmented
                                # 3D PSUM writes that cost more than the 0.2%
                                # of columns they save)
                                orow, nr = 0, ROWS
                                if trim:
                                    if r == 0 and ky == 0:
                                        orow, nr = 1, ROWS - 1
                                    if r == RT - 1 and ky == 2:
                                        nr = ROWS - 1
                                wr = prow + orow
                                if r < 4:
                                    rhs = xqt[b][:, wr : wr + nr, kx : kx + W]
                                else:
                                    rhs = xqb[b][:, wr - 32 : wr - 32 + nr,
                                                 kx : kx + W]
                                out = pts[t][:, orow * W : (orow + nr) * W]
                                nc.tensor.matmul(
                                    out, lhsT, rhs,
                                    start=(ti == 0), stop=(ti == NTAP - 1),
                                )
                        # epilogues ride ScalarE so VectorE keeps its budget
                        # for quantization (DVE is the second-busiest engine
                        # when the PE streams at full rate); the final two
                        # groups alternate ACT/DVE to parallelize the tail
                        tail_grp = g >= len(sizes) - 2
                        for t, (b, r) in enumerate(grp):
                            ot = outp.tile([128, ROWS * W], F32, tag="ot",
                                           name=f"ot{g}_{c}_{t}")
                            # edge row-tiles: the trimmed tap rows never saw
                            # the SHIFT-pad contribution, so their first/last
                            # output row takes a different effective bias
                            if trim and r == 0:
                                segs = [(0, W, bias_r0[c]),
                                        (W, ROWS * W, bias_full[c])]
                            elif trim and r == RT - 1:
                                segs = [(0, (ROWS - 1) * W, bias_full[c]),
                                        ((ROWS - 1) * W, ROWS * W, bias_r6[c])]
                            else:
                                segs = [(0, ROWS * W, bias_full[c])]
                            for s0, s1, bseg in segs:
                                if not tail_grp or t % 2 == 0:
                                    nc.scalar.activation(
                                        ot[:, s0:s1], pts[t][:, s0:s1],
                                        mybir.ActivationFunctionType.Identity,
                                        bias=bseg[:], scale=RESCALE,
                                    )
                                else:
                                    nc.vector.tensor_scalar(
                                        ot[:, s0:s1], pts[t][:, s0:s1],
                                        RESCALE, bseg[:],
                                        mybir.AluOpType.mult,
                                        mybir.AluOpType.add,
                                    )
                            if not no_out:
                                nc.scalar.dma_start(
                                    out=o_t[b, c, :, r * ROWS * W : (r + 1) * ROWS * W],
                                    in_=ot[:],
                                )

            if reps == 1:
                body()
            else:
                with tc.For_i(0, reps, 1):
                    body()
    nc.compile()
    return nc


def _get_nc():
    global _NC
    if _NC is None:
        _NC = _build()
    return _NC


def prep_in_maps(x: np.ndarray, weight: np.ndarray, bias: np.ndarray):
    """Host-side layout permutes + fp16 downcast of x (the quantizer only
    needs ~8 significant bits; fp16's 11 keep the off-by-one rate ~2%)."""
    x = np.ascontiguousarray(np.asarray(x), dtype=np.float32).reshape(
        B, CIN, NPIX).astype(np.float16)
    # pure layout permute: [co, ci, ky, kx] -> [ky*kx, ci, co]
    w_l = np.ascontiguousarray(
        np.asarray(weight, dtype=np.float32).transpose(2, 3, 1, 0)
    ).reshape(NTAP, CIN, COUT)
    b_l = np.ascontiguousarray(
        np.asarray(bias, dtype=np.float32)).reshape(NCHUNK, 128, 1)
    return [
        {
            "x": np.ascontiguousarray(x[i * BPC : (i + 1) * BPC]),
            "weight": w_l,
            "bias": b_l,
        }
        for i in range(NCORES)
    ]


def kernel(x: np.ndarray, weight: np.ndarray, bias: np.ndarray) -> np.ndarray:
    """Full inputs in, full output out. Shards batch across 8 cores."""
    nc = _get_nc()
    in_maps = prep_in_maps(x, weight, bias)
    res = run_bass_kernel_spmd(nc, in_maps, core_ids=list(range(NCORES)))
    out = np.concatenate(
        [r["out"].reshape(BPC, COUT, H, W) for r in res.results], axis=0
    )
    return out

